# revision 3
# baseline (speedup 1.0000x reference)
"""Trainium2 Bass kernel for nn_CGWeight: weighted Clebsch-Gordan tensor product.

out[k] = nan_to_num( sum_c w_c * sum_{i,j} CG_c[i,j,k] * A[ai_c][i] * H[hi_c][j] )

Folded into one small matmul against a precomputed [75, 450] constant:
  lhsT [75, 3]  = block-diag placement of concat(A) (rows 0-24, col 0),
                  concat(H) (rows 25-49, col 1), weight (rows 50-74, col 2)
  rhs  [75, 450] const G: rows 0-24 CG contraction, rows 25-49 H-scatter (0/1),
                  rows 50-74 w-scatter (0/1); columns ordered (k, c, j)
  PSUM [3, 450]: row0 = sum_i CG_c[i,j,k] A_i, row1 = H[hi_c][j], row2 = w_c
then prod = row0*row1*row2 and out[k] = segment-sum of 90-wide blocks.

The problem is far too small to shard: all 8 cores run the same replicated
kernel (SPMD), core 0's output is returned.
"""

import numpy as np
from math import factorial, sqrt

import concourse.bacc as bacc
import concourse.tile as tile
import concourse.mybir as mybir
from concourse.bass_utils import run_bass_kernel_spmd

# ---------------------------------------------------------------------------
# Compile-time constants (Clebsch-Gordan coefficients, e3nn real basis)
# ---------------------------------------------------------------------------
INPUT_A_L = [0, 1, 2, 3, 4]
INPUT_H_L = [0, 1, 2, 3, 4]
L_OUT = 2
KO = 2 * L_OUT + 1  # 5


def _su2_cg(j1, m1, j2, m2, j3, m3):
    if m3 != m1 + m2:
        return 0.0
    f = factorial
    vmin = int(max(-j1 + j2 + m3, -j1 + m1, 0))
    vmax = int(min(j2 + j3 + m1, j3 - j1 + j2, j3 + m3))
    C = sqrt((2 * j3 + 1) * f(j3 + j1 - j2) * f(j3 - j1 + j2) * f(j1 + j2 - j3) / f(j1 + j2 + j3 + 1)
             * f(j3 + m3) * f(j3 - m3) / (f(j1 + m1) * f(j1 - m1) * f(j2 + m2) * f(j2 - m2)))
    S = 0.0
    for v in range(vmin, vmax + 1):
        S += (-1) ** (v + j2 + m2) * f(j2 + j3 + m1 - v) * f(j1 - m1 + v) / (
            f(v) * f(j3 - j1 + j2 - v) * f(j3 + m3 - v) * f(v + j1 - j2 - m3))
    return C * S


def _su2_clebsch_gordan(j1, j2, j3):
    C = np.zeros((2 * j1 + 1, 2 * j2 + 1, 2 * j3 + 1))
    for m1 in range(-j1, j1 + 1):
        for m2 in range(-j2, j2 + 1):
            m3 = m1 + m2
            if -j3 <= m3 <= j3:
                C[j1 + m1, j2 + m2, j3 + m3] = _su2_cg(j1, m1, j2, m2, j3, m3)
    return C


def _change_basis_real_to_complex(l):
    q = np.zeros((2 * l + 1, 2 * l + 1), dtype=np.complex128)
    for m in range(-l, 0):
        q[l + m, l + abs(m)] = 1.0 / sqrt(2)
        q[l + m, l - abs(m)] = -1j / sqrt(2)
    q[l, l] = 1.0
    for m in range(1, l + 1):
        q[l + m, l + abs(m)] = (-1) ** m / sqrt(2)
        q[l + m, l - abs(m)] = 1j * (-1) ** m / sqrt(2)
    return (-1j) ** l * q


def _so3_clebsch_gordan(l1, l2, l3):
    C = _su2_clebsch_gordan(l1, l2, l3).astype(np.complex128)
    Q1 = _change_basis_real_to_complex(l1)
    Q2 = _change_basis_real_to_complex(l2)
    Q3 = _change_basis_real_to_complex(l3)
    C = np.einsum('ij,kl,mn,ikn->jlm', Q1, Q2, np.conj(Q3.T), C)
    assert np.abs(C.imag).max() < 1e-10
    return C.real


def _build_combos():
    combos, cgs = [], []
    for ai, la in enumerate(INPUT_A_L):
        for hi, lh in enumerate(INPUT_H_L):
            if abs(la - lh) <= L_OUT <= la + lh:
                cg = _so3_clebsch_gordan(la, lh, L_OUT)
                if np.abs(cg).sum() > 0:
                    combos.append((ai, hi))
                    cgs.append(cg.astype(np.float32))
    return combos, cgs


VALID_COMBOS, CG_TENSORS = _build_combos()
CJ_OFF = []
NCJ = 0
for (_ai, _hi) in VALID_COMBOS:
    CJ_OFF.append(NCJ)
    NCJ += 2 * _hi + 1
NCOL = KO * NCJ  # 450


def _build_G():
    G = np.zeros((75, NCOL), dtype=np.float32)
    for c, (ai, hi) in enumerate(VALID_COMBOS):
        cg = CG_TENSORS[c]
        na, nh = 2 * ai + 1, 2 * hi + 1
        for k in range(KO):
            base = k * NCJ + CJ_OFF[c]
            G[ai * ai: ai * ai + na, base: base + nh] += cg[:, :, k]
            for j in range(nh):
                G[25 + hi * hi + j, base + j] = 1.0
                G[50 + c, base + j] = 1.0
    return G


G_CONST = _build_G()

# ---------------------------------------------------------------------------
# Bass kernel
# ---------------------------------------------------------------------------
F32 = mybir.dt.float32
_NC = None


def build_nc():
    global _NC
    if _NC is not None:
        return _NC
    nc = bacc.Bacc(None, target_bir_lowering=False)

    a_dram = [nc.dram_tensor(f"a{i}", [2 * l + 1], F32, kind="ExternalInput")
              for i, l in enumerate(INPUT_A_L)]
    h_dram = [nc.dram_tensor(f"h{i}", [2 * l + 1], F32, kind="ExternalInput")
              for i, l in enumerate(INPUT_H_L)]
    w_dram = nc.dram_tensor("weight", [25], F32, kind="ExternalInput")
    g_dram = nc.dram_tensor("gmat", [75, NCOL], F32, kind="ExternalInput")
    out_dram = nc.dram_tensor("out", [KO], F32, kind="ExternalOutput")

    with tile.TileContext(nc) as tc:
        with (
            tc.tile_pool(name="pool", bufs=1) as pool,
            tc.tile_pool(name="psum", bufs=1, space="PSUM") as psum_pool,
        ):
            ctile = pool.tile([75, NCOL], F32)
            nc.sync.dma_start(ctile[:], g_dram[:])

            # A/H/w go in columns 0/32/64 so the matmul's result rows land on
            # partitions 0/32/64 (compute-engine APs must start on 0/32/64/96).
            itile = pool.tile([75, 65], F32)
            nc.vector.memset(itile[:], 0.0)
            for i, l in enumerate(INPUT_A_L):
                nc.sync.dma_start(itile[l * l:(l + 1) * (l + 1), 0:1], a_dram[i][:])
            for i, l in enumerate(INPUT_H_L):
                nc.sync.dma_start(itile[25 + l * l:25 + (l + 1) * (l + 1), 32:33], h_dram[i][:])
            nc.sync.dma_start(itile[50:75, 64:65], w_dram[:])

            acc = psum_pool.tile([65, NCOL], F32)
            nc.tensor.matmul(acc[:], itile[:], ctile[:], start=True, stop=True)

            s0 = pool.tile([1, NCOL], F32)
            nc.vector.tensor_copy(s0[:], acc[0:1, :])
            t1 = pool.tile([1, NCOL], F32)
            nc.vector.tensor_mul(t1[:], s0[:], acc[32:33, :])
            t2 = pool.tile([1, NCOL], F32)
            nc.vector.tensor_mul(t2[:], t1[:], acc[64:65, :])
            o5 = pool.tile([1, KO], F32)
            nc.vector.reduce_sum(
                o5[:], t2[:].rearrange("p (a b) -> p a b", b=NCJ),
                axis=mybir.AxisListType.X)
            nc.sync.dma_start(out_dram[:], o5[:])

    nc.compile()
    _NC = nc
    return nc


def kernel(**inputs) -> np.ndarray:
    nc = build_nc()
    in_map = {k: np.ascontiguousarray(np.asarray(v, dtype=np.float32))
              for k, v in inputs.items()}
    in_map["gmat"] = G_CONST
    core_ids = list(range(8))
    res = run_bass_kernel_spmd(nc, [dict(in_map) for _ in core_ids], core_ids)
    return res.results[0]["out"]


# revision 8
# speedup vs baseline: 1.3713x; 1.3713x over previous
"""Trainium2 Bass kernel for nn_CGWeight: weighted Clebsch-Gordan tensor product.

out[k] = nan_to_num( sum_c w_c * sum_{i,j} CG_c[i,j,k] * A[ai_c][i] * H[hi_c][j] )

One small fp32 matmul against a precomputed [75, 450] constant G:
  lhsT [75, 65]  = host-packed block-diagonal: concat(A) rows 0-24 col 0,
                   concat(H) rows 25-49 col 32, weight rows 50-74 col 64
                   (cols 0/32/64 so result rows hit legal DVE start partitions)
  rhs  [75, 450] = G: rows 0-24 CG contraction (cols ordered (k, c, j)),
                   rows 25-49 H-scatter 0/1, rows 50-74 w-scatter 0/1
  PSUM [65, 450]: row0 = B[(k,c,j)] = sum_i CG_c[i,j,k] A_i,
                  row32 = H[hi_c][j] (k-periodic), row64 = w_c (k-periodic)
then hw = row32*row64 (first 90 cols), prod = B * broadcast(hw), out[k] =
segment-sum of 90-wide blocks.  Raw Bass (no TileContext) with manual
semaphores: DMAs spread across engine queues, minimal fixed overhead.

Too small to shard: all 8 cores run the same replicated program (SPMD);
core 0's output is returned.
"""

from contextlib import ExitStack
from math import factorial, sqrt

import numpy as np

import concourse.bass as bass
import concourse.mybir as mybir
from concourse.bass_utils import run_bass_kernel_spmd

# ---------------------------------------------------------------------------
# Compile-time constants (Clebsch-Gordan coefficients, e3nn real basis)
# ---------------------------------------------------------------------------
INPUT_A_L = [0, 1, 2, 3, 4]
INPUT_H_L = [0, 1, 2, 3, 4]
L_OUT = 2
KO = 2 * L_OUT + 1  # 5


def _su2_cg(j1, m1, j2, m2, j3, m3):
    if m3 != m1 + m2:
        return 0.0
    f = factorial
    vmin = int(max(-j1 + j2 + m3, -j1 + m1, 0))
    vmax = int(min(j2 + j3 + m1, j3 - j1 + j2, j3 + m3))
    C = sqrt((2 * j3 + 1) * f(j3 + j1 - j2) * f(j3 - j1 + j2) * f(j1 + j2 - j3) / f(j1 + j2 + j3 + 1)
             * f(j3 + m3) * f(j3 - m3) / (f(j1 + m1) * f(j1 - m1) * f(j2 + m2) * f(j2 - m2)))
    S = 0.0
    for v in range(vmin, vmax + 1):
        S += (-1) ** (v + j2 + m2) * f(j2 + j3 + m1 - v) * f(j1 - m1 + v) / (
            f(v) * f(j3 - j1 + j2 - v) * f(j3 + m3 - v) * f(v + j1 - j2 - m3))
    return C * S


def _su2_clebsch_gordan(j1, j2, j3):
    C = np.zeros((2 * j1 + 1, 2 * j2 + 1, 2 * j3 + 1))
    for m1 in range(-j1, j1 + 1):
        for m2 in range(-j2, j2 + 1):
            m3 = m1 + m2
            if -j3 <= m3 <= j3:
                C[j1 + m1, j2 + m2, j3 + m3] = _su2_cg(j1, m1, j2, m2, j3, m3)
    return C


def _change_basis_real_to_complex(l):
    q = np.zeros((2 * l + 1, 2 * l + 1), dtype=np.complex128)
    for m in range(-l, 0):
        q[l + m, l + abs(m)] = 1.0 / sqrt(2)
        q[l + m, l - abs(m)] = -1j / sqrt(2)
    q[l, l] = 1.0
    for m in range(1, l + 1):
        q[l + m, l + abs(m)] = (-1) ** m / sqrt(2)
        q[l + m, l - abs(m)] = 1j * (-1) ** m / sqrt(2)
    return (-1j) ** l * q


def _so3_clebsch_gordan(l1, l2, l3):
    C = _su2_clebsch_gordan(l1, l2, l3).astype(np.complex128)
    Q1 = _change_basis_real_to_complex(l1)
    Q2 = _change_basis_real_to_complex(l2)
    Q3 = _change_basis_real_to_complex(l3)
    C = np.einsum('ij,kl,mn,ikn->jlm', Q1, Q2, np.conj(Q3.T), C)
    assert np.abs(C.imag).max() < 1e-10
    return C.real


def _build_combos():
    combos, cgs = [], []
    for ai, la in enumerate(INPUT_A_L):
        for hi, lh in enumerate(INPUT_H_L):
            if abs(la - lh) <= L_OUT <= la + lh:
                cg = _so3_clebsch_gordan(la, lh, L_OUT)
                if np.abs(cg).sum() > 0:
                    combos.append((ai, hi))
                    cgs.append(cg.astype(np.float32))
    return combos, cgs


VALID_COMBOS, CG_TENSORS = _build_combos()
CJ_OFF = []
NCJ = 0
for (_ai, _hi) in VALID_COMBOS:
    CJ_OFF.append(NCJ)
    NCJ += 2 * _hi + 1
NCOL = KO * NCJ  # 450


def _build_G():
    G = np.zeros((75, NCOL), dtype=np.float32)
    for c, (ai, hi) in enumerate(VALID_COMBOS):
        cg = CG_TENSORS[c]
        na, nh = 2 * ai + 1, 2 * hi + 1
        for k in range(KO):
            base = k * NCJ + CJ_OFF[c]
            G[ai * ai: ai * ai + na, base: base + nh] += cg[:, :, k]
            for j in range(nh):
                G[25 + hi * hi + j, base + j] = 1.0
                G[50 + c, base + j] = 1.0
    return G


G_CONST = _build_G()

# ---------------------------------------------------------------------------
# Bass kernel (raw, manual semaphores)
# ---------------------------------------------------------------------------
F32 = mybir.dt.float32
_NC = None


def build_nc():
    global _NC
    if _NC is not None:
        return _NC
    nc = bass.Bass(target_bir_lowering=False)

    iv = nc.dram_tensor("iv", [75, 65], F32, kind="ExternalInput")
    gm = nc.dram_tensor("gmat", [75, NCOL], F32, kind="ExternalInput")
    out_dram = nc.dram_tensor("out", [KO], F32, kind="ExternalOutput")

    with ExitStack() as ctx:
        itile = ctx.enter_context(nc.sbuf_tensor("itile", [75, 65], F32))
        ctile = ctx.enter_context(nc.sbuf_tensor("ctile", [75, NCOL], F32))
        s1 = ctx.enter_context(nc.sbuf_tensor("s1", [1, NCJ], F32))
        hw = ctx.enter_context(nc.sbuf_tensor("hw", [1, NCJ], F32))
        prod = ctx.enter_context(nc.sbuf_tensor("prod", [1, NCOL], F32))
        o5 = ctx.enter_context(nc.sbuf_tensor("o5", [1, KO], F32))
        acc = ctx.enter_context(nc.psum_tensor("acc", [65, NCOL], F32))
        s_iv = ctx.enter_context(nc.semaphore("s_iv"))
        s_g = ctx.enter_context(nc.semaphore("s_g"))
        s_g2 = ctx.enter_context(nc.semaphore("s_g2"))
        s_mm = ctx.enter_context(nc.semaphore("s_mm"))
        s_ve = ctx.enter_context(nc.semaphore("s_ve"))
        s_out = ctx.enter_context(nc.semaphore("s_out"))
        block = ctx.enter_context(nc.Block())

        @block.sync
        def _(sync):
            sync.dma_start(itile[:], iv[:]).then_inc(s_iv, 16)
            sync.dma_start(ctile[0:25, :], gm[0:25, :]).then_inc(s_g, 16)
            sync.wait_ge(s_ve, 4)
            sync.dma_start(out_dram[:], o5[:]).then_inc(s_out, 16)
            sync.wait_ge(s_out, 16)

        @block.gpsimd
        def _(gpsimd):
            gpsimd.dma_start(ctile[25:50, :], gm[25:50, :]).then_inc(s_g2, 16)

        @block.scalar
        def _(scalar):
            scalar.dma_start(ctile[50:75, :], gm[50:75, :]).then_inc(s_g, 16)

        @block.tensor
        def _(tensor):
            tensor.wait_ge(s_iv, 16)
            tensor.wait_ge(s_g, 32)
            tensor.wait_ge(s_g2, 16)
            nc.tensor.matmul(acc[:], itile[:], ctile[:],
                             start=True, stop=True).then_inc(s_mm, 1)

        @block.vector
        def _(vector):
            vector.wait_ge(s_mm, 1)
            nc.vector.tensor_copy(s1[:], acc[32:33, 0:NCJ]).then_inc(s_ve, 1)
            vector.wait_ge(s_ve, 1)
            nc.vector.tensor_mul(hw[:], s1[:], acc[64:65, 0:NCJ]).then_inc(s_ve, 1)
            hw_bcast = bass.AP(hw, 0, [[NCJ, 1], [0, KO], [1, NCJ]])
            vector.wait_ge(s_ve, 2)
            nc.vector.tensor_mul(
                prod[:].rearrange("p (a b) -> p a b", b=NCJ),
                acc[0:1, :].rearrange("p (a b) -> p a b", b=NCJ),
                hw_bcast).then_inc(s_ve, 1)
            vector.wait_ge(s_ve, 3)
            nc.vector.reduce_sum(
                o5[:], prod[:].rearrange("p (a b) -> p a b", b=NCJ),
                axis=mybir.AxisListType.X).then_inc(s_ve, 1)

    _NC = nc
    return nc


def _pack_iv(inputs) -> np.ndarray:
    iv = np.zeros((75, 65), dtype=np.float32)
    for i, l in enumerate(INPUT_A_L):
        iv[l * l:(l + 1) * (l + 1), 0] = np.asarray(inputs[f"a{i}"], np.float32)
    for i, l in enumerate(INPUT_H_L):
        iv[25 + l * l:25 + (l + 1) * (l + 1), 32] = np.asarray(inputs[f"h{i}"], np.float32)
    iv[50:75, 64] = np.asarray(inputs["weight"], np.float32)
    return iv


def kernel(**inputs) -> np.ndarray:
    nc = build_nc()
    in_map = {"iv": _pack_iv(inputs), "gmat": G_CONST}
    core_ids = list(range(8))
    res = run_bass_kernel_spmd(nc, [dict(in_map) for _ in core_ids], core_ids)
    return res.results[0]["out"]


# revision 9
# speedup vs baseline: 1.5729x; 1.1470x over previous
"""Trainium2 Bass kernel for nn_CGWeight: weighted Clebsch-Gordan tensor product.

out[k] = nan_to_num( sum_c w_c * sum_{i,j} CG_c[i,j,k] * A[ai_c][i] * H[hi_c][j] )

One small fp32 matmul against a precomputed [75, 450] constant G:
  lhsT [75, 65]  = host-packed block-diagonal: concat(A) rows 0-24 col 0,
                   concat(H) rows 25-49 col 32, weight rows 50-74 col 64
                   (cols 0/32/64 so result rows hit legal DVE start partitions)
  rhs  [75, 450] = G: rows 0-24 CG contraction (cols ordered (k, c, j)),
                   rows 25-49 H-scatter 0/1, rows 50-74 w-scatter 0/1
  PSUM [65, 450]: row0 = B[(k,c,j)] = sum_i CG_c[i,j,k] A_i,
                  row32 = H[hi_c][j] (k-periodic), row64 = w_c (k-periodic)
then hw = row32*row64 (first 90 cols), prod = B * broadcast(hw), out[k] =
segment-sum of 90-wide blocks.  Raw Bass (no TileContext) with manual
semaphores: DMAs spread across engine queues, minimal fixed overhead.

Too small to shard: all 8 cores run the same replicated program (SPMD);
core 0's output is returned.
"""

from contextlib import ExitStack
from math import factorial, sqrt

import numpy as np

import concourse.bass as bass
import concourse.mybir as mybir
from concourse.bass_utils import run_bass_kernel_spmd

# ---------------------------------------------------------------------------
# Compile-time constants (Clebsch-Gordan coefficients, e3nn real basis)
# ---------------------------------------------------------------------------
INPUT_A_L = [0, 1, 2, 3, 4]
INPUT_H_L = [0, 1, 2, 3, 4]
L_OUT = 2
KO = 2 * L_OUT + 1  # 5


def _su2_cg(j1, m1, j2, m2, j3, m3):
    if m3 != m1 + m2:
        return 0.0
    f = factorial
    vmin = int(max(-j1 + j2 + m3, -j1 + m1, 0))
    vmax = int(min(j2 + j3 + m1, j3 - j1 + j2, j3 + m3))
    C = sqrt((2 * j3 + 1) * f(j3 + j1 - j2) * f(j3 - j1 + j2) * f(j1 + j2 - j3) / f(j1 + j2 + j3 + 1)
             * f(j3 + m3) * f(j3 - m3) / (f(j1 + m1) * f(j1 - m1) * f(j2 + m2) * f(j2 - m2)))
    S = 0.0
    for v in range(vmin, vmax + 1):
        S += (-1) ** (v + j2 + m2) * f(j2 + j3 + m1 - v) * f(j1 - m1 + v) / (
            f(v) * f(j3 - j1 + j2 - v) * f(j3 + m3 - v) * f(v + j1 - j2 - m3))
    return C * S


def _su2_clebsch_gordan(j1, j2, j3):
    C = np.zeros((2 * j1 + 1, 2 * j2 + 1, 2 * j3 + 1))
    for m1 in range(-j1, j1 + 1):
        for m2 in range(-j2, j2 + 1):
            m3 = m1 + m2
            if -j3 <= m3 <= j3:
                C[j1 + m1, j2 + m2, j3 + m3] = _su2_cg(j1, m1, j2, m2, j3, m3)
    return C


def _change_basis_real_to_complex(l):
    q = np.zeros((2 * l + 1, 2 * l + 1), dtype=np.complex128)
    for m in range(-l, 0):
        q[l + m, l + abs(m)] = 1.0 / sqrt(2)
        q[l + m, l - abs(m)] = -1j / sqrt(2)
    q[l, l] = 1.0
    for m in range(1, l + 1):
        q[l + m, l + abs(m)] = (-1) ** m / sqrt(2)
        q[l + m, l - abs(m)] = 1j * (-1) ** m / sqrt(2)
    return (-1j) ** l * q


def _so3_clebsch_gordan(l1, l2, l3):
    C = _su2_clebsch_gordan(l1, l2, l3).astype(np.complex128)
    Q1 = _change_basis_real_to_complex(l1)
    Q2 = _change_basis_real_to_complex(l2)
    Q3 = _change_basis_real_to_complex(l3)
    C = np.einsum('ij,kl,mn,ikn->jlm', Q1, Q2, np.conj(Q3.T), C)
    assert np.abs(C.imag).max() < 1e-10
    return C.real


def _build_combos():
    combos, cgs = [], []
    for ai, la in enumerate(INPUT_A_L):
        for hi, lh in enumerate(INPUT_H_L):
            if abs(la - lh) <= L_OUT <= la + lh:
                cg = _so3_clebsch_gordan(la, lh, L_OUT)
                if np.abs(cg).sum() > 0:
                    combos.append((ai, hi))
                    cgs.append(cg.astype(np.float32))
    return combos, cgs


VALID_COMBOS, CG_TENSORS = _build_combos()
CJ_OFF = []
NCJ = 0
for (_ai, _hi) in VALID_COMBOS:
    CJ_OFF.append(NCJ)
    NCJ += 2 * _hi + 1
NCOL = KO * NCJ  # 450


def _build_G():
    G = np.zeros((75, NCOL), dtype=np.float32)
    for c, (ai, hi) in enumerate(VALID_COMBOS):
        cg = CG_TENSORS[c]
        na, nh = 2 * ai + 1, 2 * hi + 1
        for k in range(KO):
            base = k * NCJ + CJ_OFF[c]
            G[ai * ai: ai * ai + na, base: base + nh] += cg[:, :, k]
            for j in range(nh):
                G[25 + hi * hi + j, base + j] = 1.0
                G[50 + c, base + j] = 1.0
    return G


G_CONST = _build_G()

# ---------------------------------------------------------------------------
# Bass kernel (raw, manual semaphores)
# ---------------------------------------------------------------------------
F32 = mybir.dt.float32
F16 = mybir.dt.float16
_NC = None


def build_nc():
    global _NC
    if _NC is not None:
        return _NC
    nc = bass.Bass(target_bir_lowering=False)

    iv = nc.dram_tensor("iv", [75, 65], F16, kind="ExternalInput")
    gm = nc.dram_tensor("gmat", [75, NCOL], F16, kind="ExternalInput")
    out_dram = nc.dram_tensor("out", [KO], F32, kind="ExternalOutput")

    with ExitStack() as ctx:
        itile = ctx.enter_context(nc.sbuf_tensor("itile", [75, 65], F16))
        ctile = ctx.enter_context(nc.sbuf_tensor("ctile", [75, NCOL], F16))
        s1 = ctx.enter_context(nc.sbuf_tensor("s1", [1, NCJ], F32))
        hw = ctx.enter_context(nc.sbuf_tensor("hw", [1, NCJ], F32))
        prod = ctx.enter_context(nc.sbuf_tensor("prod", [1, NCOL], F32))
        o5 = ctx.enter_context(nc.sbuf_tensor("o5", [1, KO], F32))
        acc = ctx.enter_context(nc.psum_tensor("acc", [65, NCOL], F32))
        s_iv = ctx.enter_context(nc.semaphore("s_iv"))
        s_g = ctx.enter_context(nc.semaphore("s_g"))
        s_g2 = ctx.enter_context(nc.semaphore("s_g2"))
        s_mm = ctx.enter_context(nc.semaphore("s_mm"))
        s_ve = ctx.enter_context(nc.semaphore("s_ve"))
        s_out = ctx.enter_context(nc.semaphore("s_out"))
        block = ctx.enter_context(nc.Block())

        @block.sync
        def _(sync):
            sync.dma_start(itile[:], iv[:]).then_inc(s_iv, 16)
            sync.dma_start(ctile[0:25, :], gm[0:25, :]).then_inc(s_g, 16)
            sync.wait_ge(s_ve, 4)
            sync.dma_start(out_dram[:], o5[:]).then_inc(s_out, 16)
            sync.wait_ge(s_out, 16)

        @block.gpsimd
        def _(gpsimd):
            gpsimd.dma_start(ctile[25:50, :], gm[25:50, :]).then_inc(s_g2, 16)

        @block.scalar
        def _(scalar):
            scalar.dma_start(ctile[50:75, :], gm[50:75, :]).then_inc(s_g, 16)

        @block.tensor
        def _(tensor):
            tensor.wait_ge(s_iv, 16)
            tensor.wait_ge(s_g, 32)
            tensor.wait_ge(s_g2, 16)
            nc.tensor.matmul(acc[:], itile[:], ctile[:],
                             start=True, stop=True).then_inc(s_mm, 1)

        @block.vector
        def _(vector):
            nc.vector.tensor_copy(s1[:], acc[32:33, 0:NCJ]) \
                .wait_op(s_mm, 1, "sem-ge").then_inc(s_ve, 1)
            nc.vector.tensor_mul(hw[:], s1[:], acc[64:65, 0:NCJ]) \
                .wait_op(s_ve, 1, "sem-ge").then_inc(s_ve, 1)
            hw_bcast = bass.AP(hw, 0, [[NCJ, 1], [0, KO], [1, NCJ]])
            nc.vector.tensor_mul(
                prod[:].rearrange("p (a b) -> p a b", b=NCJ),
                acc[0:1, :].rearrange("p (a b) -> p a b", b=NCJ),
                hw_bcast).wait_op(s_ve, 2, "sem-ge").then_inc(s_ve, 1)
            nc.vector.reduce_sum(
                o5[:], prod[:].rearrange("p (a b) -> p a b", b=NCJ),
                axis=mybir.AxisListType.X) \
                .wait_op(s_ve, 3, "sem-ge").then_inc(s_ve, 1)

    _NC = nc
    return nc


def _pack_iv(inputs) -> np.ndarray:
    iv = np.zeros((75, 65), dtype=np.float16)
    for i, l in enumerate(INPUT_A_L):
        iv[l * l:(l + 1) * (l + 1), 0] = np.asarray(inputs[f"a{i}"], np.float16)
    for i, l in enumerate(INPUT_H_L):
        iv[25 + l * l:25 + (l + 1) * (l + 1), 32] = np.asarray(inputs[f"h{i}"], np.float16)
    iv[50:75, 64] = np.asarray(inputs["weight"], np.float16)
    return iv


def kernel(**inputs) -> np.ndarray:
    nc = build_nc()
    in_map = {"iv": _pack_iv(inputs), "gmat": G_CONST.astype(np.float16)}
    core_ids = list(range(8))
    res = run_bass_kernel_spmd(nc, [dict(in_map) for _ in core_ids], core_ids)
    return res.results[0]["out"]
